# revision 1
# baseline (speedup 1.0000x reference)
"""Trainium2 Bass kernel for nn_CauseEffectRepertoire.

Computes, for each of 2 directions (cause/effect) and batch b:
    min over masks m of KL(full_b || 0.5*(softmax(MLP(state_b*bits_m)) +
                                          softmax(MLP(state_b*(1-bits_m)))))
with D=16, H=64, B=8, M=2^15-1=32767 masks, via an 8-core SPMD kernel that
shards the mask axis (4096 masks per core, padded with one duplicate mask).

Structure (per 512-mask chunk, per direction):
  - mm1 moving operand carries [bits | 1-bits] columns, so one matmul pair
    emits both A=x_a@W and colsum(W)-A=x_b@W; with the SHARED per-partition
    bias b1 the ra/rb fork collapses to ONE wide relu per pair.
  - mm2 (32-row bands, tile_position) -> one [La | Lb] PSUM tile -> ONE wide
    exp -> E.
  - G2 (plain block mask) broadcasts [Zb | Za] to all (b,d) rows; u = Ea*Zb
    + Eb*Za in fp16 (host re-evaluates candidates exactly, so constant
    offsets and mild quantization noise are ranking-safe).
  - dot rows: dstack = fmat@ln(u); zs = sumsel@u = 2*Za*Zb; per 4-chunk
    group: sout = dstack - ln(zs).
Host: per-(b,dir) ranking via max_m s; exact float64 re-eval of candidates
within DELTA of the device max.
"""

import os
import sys
from contextlib import ExitStack

import numpy as np

sys.path.insert(0, "/opt/trn_rl_repo")

D, H, B = 16, 64, 8
M = 2 ** (D - 1) - 1  # 32767
NCORES = 8
MPAD = 32768
MC = MPAD // NCORES  # 4096 masks per core
CHUNK = 512
NCHUNK = MC // CHUNK  # 8
RELU_ENG = ("act", "dve", "act", "dve")  # fork engine per pair
USE_GPS_ADD = True  # route the u-add to GpSimd instead of VectorE
B2SHIFT = 1.5  # exp bias shift keeping zz=2*Za*Zb in fp16 range

_f32 = np.float32
_f16 = np.float16


def _mlp_softmax_np(x, w1, b1, w2, b2):
    h = np.maximum(x @ w1.T + b1, 0.0)
    lg = h @ w2.T + b2
    lg = lg - lg.max(axis=-1, keepdims=True)
    e = np.exp(lg)
    return e / e.sum(axis=-1, keepdims=True)


def _host_prep(inputs):
    """Build all device input arrays (float64 math, float32/16 outputs)."""
    state = np.asarray(inputs["state"], dtype=np.float64)  # (B, D)
    dirs = []
    for pre in ("cause", "effect"):
        dirs.append(
            tuple(
                np.asarray(inputs[f"{pre}_{k}"], dtype=np.float64)
                for k in ("w1", "b1", "w2", "b2")
            )
        )

    # mask bits, padded to MPAD with a duplicate of mask value 1
    mv = np.concatenate([np.arange(1, M + 1, dtype=np.int64), [1]])
    bits = ((mv[:, None] >> np.arange(D)[None, :]) & 1).astype(np.float64)  # (MPAD, D)

    # per-core bitsQ2 (128, 2*MC): per 512-chunk, [bits.T | (1-bits).T], each
    # band-structured: (16,512) duplicated to 32 rows, tiled 4x to 128.
    def band128(bT):  # (16, 512) -> (128, 512)
        band = np.concatenate([bT, bT], axis=0)
        return np.tile(band, (4, 1))

    bitsQ2_cores = []
    for c in range(NCORES):
        cols = []
        for n in range(NCHUNK):
            bsel = bits[c * MC + n * CHUNK : c * MC + (n + 1) * CHUNK]  # (512, D)
            cols.append(band128(bsel.T))
            cols.append(band128((1.0 - bsel).T))
        bitsQ2_cores.append(np.concatenate(cols, axis=1).astype(_f16))

    # mm1 stationaries (fp16): (128, 8*128); pair p in row band 32p
    mm1w = np.zeros((128, 8 * 128))
    b1t = np.zeros((128, 2))
    for d_ in range(2):
        w1, b1, w2, b2 = dirs[d_]
        b1t[0:64, d_] = b1
        b1t[64:128, d_] = b1
        for p in range(4):
            idx = d_ * 4 + p
            for half, b_ in enumerate((2 * p, 2 * p + 1)):
                Wb = state[b_][:, None] * w1.T  # (D, H)
                r0 = 32 * p + half * 16
                c0 = idx * 128 + half * 64
                mm1w[r0 : r0 + 16, c0 : c0 + 64] = Wb

    # mm2 stationaries (fp16): per dir one (128, 32) block-diag +w2T
    mm2w = np.zeros((128, 64))
    for d_ in range(2):
        w2T = dirs[d_][2].T  # (H, D)
        c0 = d_ * 32
        mm2w[0:64, c0 : c0 + 16] = w2T
        mm2w[64:128, c0 + 16 : c0 + 32] = w2T

    # plain Z-broadcast mask G2 (dir-independent)
    jj = np.arange(128)
    G2 = (jj[:, None] // 16 == jj[None, :] // 16).astype(np.float64)

    # b2 shifted by -B2SHIFT: scales E by e^-B2SHIFT so fp16 intermediates
    # stay in range; a uniform shift only offsets s per (b,dir).
    b2t = np.zeros((128, 2))
    for d_ in range(2):
        b2t[:, d_] = np.tile(dirs[d_][3], B) - B2SHIFT

    shared = {
        "mm1w": mm1w.astype(_f16),
        "b1t": b1t.astype(_f32),
        "mm2w": mm2w.astype(_f16),
        "G2": G2.astype(_f16),
        "b2t": b2t.astype(_f32),
    }
    in_maps = []
    for c in range(NCORES):
        m = dict(shared)
        m["bitsQ2"] = bitsQ2_cores[c]
        in_maps.append(m)
    return in_maps, bits, dirs, state


def _patch_act_tables():
    """Force every activation to resolve to natural_log_exp_and_others
    (contains Ln, Exp, Relu, Copy, Identity) so the kernel pays exactly one
    ACT table load instead of one per Exp<->Ln<->Relu transition (~2.7us
    each). Set indices are preserved; other sets are just made unmatchable.
    """
    import concourse.bacc as bacc_mod
    from concourse import hw_specs

    if getattr(bacc_mod, "_act_tables_patched", False):
        return
    orig = hw_specs.get_activation_tables

    def only_nle(arch):
        t = dict(orig(arch))
        if "natural_log_exp_and_others" in t:
            t = {
                k: (v if k == "natural_log_exp_and_others" else set())
                for k, v in t.items()
            }
        return t

    bacc_mod.get_activation_tables = only_nle
    bacc_mod._act_tables_patched = True


_NC_CACHE = {}


def build_nc(repeats=1):
    """Build and compile the 8-core SPMD Bass program (cached).

    repeats>1 wraps the whole computation in a device-side loop — used only
    for benchmarking (amortizes host/tunnel dispatch overhead).
    """
    if repeats in _NC_CACHE:
        return _NC_CACHE[repeats]

    import concourse.bacc as bacc
    import concourse.bass as bass
    import concourse.tile as tile
    from concourse import mybir

    _patch_act_tables()

    AF = mybir.ActivationFunctionType
    OP = mybir.AluOpType
    f32 = mybir.dt.float32
    f16 = mybir.dt.float16

    nc = bacc.Bacc(
        "TRN2", target_bir_lowering=False, debug=False, num_devices=NCORES
    )

    ins = {}
    dts = {}
    for name, shape, dt in (
        ("bitsQ2", (128, 2 * MC), f16),
        ("mm1w", (128, 8 * 128), f16),
        ("b1t", (128, 2), f32),
        ("mm2w", (128, 64), f16),
        ("G2", (128, 128), f16),
        ("b2t", (128, 2), f32),
    ):
        ins[name] = nc.dram_tensor(name, shape, dt, kind="ExternalInput").ap()
        dts[name] = dt
    # raw u' dump: (128, 16*CHUNK): col block (d*NCHUNK+n)*CHUNK holds
    # chunk n of dir d; host computes s = fmat.ln(u) - ln(sum_d u) itself
    out_d = nc.dram_tensor("sdump", (128, 2 * MC), f16,
                           kind="ExternalOutput").ap()

    with tile.TileContext(nc) as tc, ExitStack() as ctx:
        cpool = ctx.enter_context(tc.tile_pool(name="consts", bufs=1))
        spool = ctx.enter_context(tc.tile_pool(name="work", bufs=2))
        rpool = ctx.enter_context(tc.tile_pool(name="relu", bufs=4))
        # PSUM (8 banks): l1 2x2 + L2 2x2; the Z broadcast overwrites L2 in
        # place, so L2(n+1) only waits exp(n) and the mid ring breaks
        pp_l1 = ctx.enter_context(tc.tile_pool(name="pl1", bufs=2, space="PSUM"))
        pp_mid = ctx.enter_context(tc.tile_pool(name="pmid", bufs=2, space="PSUM"))

        ct = {}
        for name in ins:
            shp = list(ins[name].shape)
            t = cpool.tile(shp, dts[name], tag=name, name=f"c_{name}")
            nc.sync.dma_start(t[:], ins[name][:])
            ct[name] = t

        rep_ctx = tc.For_i(0, repeats, 1) if repeats > 1 else None
        if rep_ctx is not None:
            rep_ctx.__enter__()

        # Software-pipelined: iteration k emits chunk k's HEAD (mm1 + relu +
        # mm2) and chunk k-1's TAIL (exp -> Z broadcast -> muls -> add ->
        # dma). G2(k-1) then sits AFTER mm1(k) in the PE queue, so the ring
        # exp(k) -> G2(k) -> mm1(k+1) -> relu(k+1) -> mm2(k+1) -> exp(k+1)
        # no longer paces the chunk period.
        chunks = [(d_, n) for d_ in range(2) for n in range(NCHUNK)]
        live = {}

        def head(k):
            d_, n = chunks[k]
            L2 = pp_mid.tile([128, 2 * CHUNK], f32, tag="L2")
            for p in range(4):
                idx = d_ * 4 + p
                lt = pp_l1.tile([128, 2 * CHUNK], f32, tag="l1")
                for h_ in range(2):  # per-bank halves (bits / 1-bits)
                    nc.tensor.matmul(
                        lt[:, h_ * CHUNK : (h_ + 1) * CHUNK],
                        ct["mm1w"][:, idx * 128 : (idx + 1) * 128],
                        ct["bitsQ2"][:, (2 * n + h_) * CHUNK :
                                     (2 * n + h_ + 1) * CHUNK],
                        start=True, stop=True,
                    )
                rall = rpool.tile([128, 2 * CHUNK], f16, tag="rall")
                if RELU_ENG[p] == "act":
                    nc.scalar.activation(
                        rall[:], lt[:], AF.Relu,
                        bias=ct["b1t"][:, d_ : d_ + 1],
                    )
                elif RELU_ENG[p] == "gps":
                    nc.gpsimd.tensor_scalar(
                        rall[:], lt[:], ct["b1t"][:, d_ : d_ + 1], 0.0,
                        OP.add, OP.max,
                    )
                else:
                    nc.vector.tensor_scalar(
                        rall[:], lt[:], ct["b1t"][:, d_ : d_ + 1], 0.0,
                        OP.add, OP.max,
                    )
                w2blk = ct["mm2w"][:, d_ * 32 : d_ * 32 + 32]
                for h_ in range(2):
                    nc.tensor.matmul(
                        L2[32 * p : 32 * p + 32,
                           h_ * CHUNK : (h_ + 1) * CHUNK],
                        w2blk,
                        rall[:, h_ * CHUNK : (h_ + 1) * CHUNK],
                        start=True, stop=True, tile_position=(0, 32 * p),
                    )
            live[k] = (d_, n, L2)

        def tail(k):
            d_, n, L2 = live.pop(k)
            # ---- one wide exp over [La | Lb] (fp16 out, bias=b2) ----
            E2 = spool.tile([128, 2 * CHUNK], f16, tag="E2")
            nc.scalar.activation(
                E2[:], L2[:], AF.Exp, bias=ct["b2t"][:, d_ : d_ + 1]
            )
            # ---- broadcast Z to (b,d) rows, overwriting L2 in place:
            # L2 <- [Zb | Za] ----
            uu = spool.tile([128, 2 * CHUNK], f16, tag="uu")
            nc.tensor.matmul(L2[:, 0:CHUNK], ct["G2"][:],
                             E2[:, CHUNK : 2 * CHUNK], start=True, stop=True)
            # ---- u' = Ea*Zb + Eb*Za; each half multiplies as soon as its
            # broadcast lands, freeing the L2 buffer earlier ----
            nc.vector.tensor_mul(uu[:, 0:CHUNK], E2[:, 0:CHUNK],
                                 L2[:, 0:CHUNK])
            nc.tensor.matmul(L2[:, CHUNK : 2 * CHUNK], ct["G2"][:],
                             E2[:, 0:CHUNK], start=True, stop=True)
            nc.vector.tensor_mul(uu[:, CHUNK : 2 * CHUNK],
                                 E2[:, CHUNK : 2 * CHUNK],
                                 L2[:, CHUNK : 2 * CHUNK])
            u = spool.tile([128, CHUNK], f16, tag="u", bufs=4)
            if USE_GPS_ADD:
                nc.gpsimd.tensor_tensor(
                    u[:], uu[:, 0:CHUNK], uu[:, CHUNK : 2 * CHUNK], OP.add
                )
            else:
                nc.vector.tensor_add(
                    u[:], uu[:, 0:CHUNK], uu[:, CHUNK : 2 * CHUNK]
                )
            # stream u straight to DRAM; host does ln/dots/ranking
            nc.sync.dma_start(
                out_d[:, (d_ * NCHUNK + n) * CHUNK :
                      (d_ * NCHUNK + n + 1) * CHUNK],
                u[:],
            )

        for k in range(len(chunks)):
            head(k)
            if k > 0:
                tail(k - 1)
        tail(len(chunks) - 1)

        if rep_ctx is not None:
            rep_ctx.__exit__(None, None, None)

    nc.compile()
    _NC_CACHE[repeats] = nc
    return nc


DELTA = 0.05  # nats: candidate margin below the device max (>> fp16 noise)


def kernel(**inputs):
    from concourse.bass_utils import run_bass_kernel_spmd

    in_maps, bits, dirs, state = _host_prep(inputs)
    nc = build_nc()
    res = run_bass_kernel_spmd(nc, in_maps, list(range(NCORES)))
    sd = np.stack([r["sdump"].astype(np.float32) for r in res.results])

    # sd: (core, 128, 2*MC); block (d*NCHUNK+n)*CHUNK = chunk n of dir d.
    # s[m, b] = sum_d full[b,d]*ln(u[b,d]) - ln(sum_d u[b,d]) + consts
    u_all = sd.reshape(NCORES, 128, 2, NCHUNK, CHUNK)
    u_all = u_all.transpose(2, 0, 3, 4, 1)  # (dir, core, n, c, 128)
    u_all = u_all.reshape(2, MPAD, B, D)  # (dir, mask, b, d)
    lnu = np.log(np.maximum(u_all, 1e-30))
    s_all = np.empty((2, MPAD, B), np.float32)
    for d_ in range(2):
        w1, b1, w2, b2 = dirs[d_]
        full = _mlp_softmax_np(state, w1, b1, w2, b2)  # (B, D)
        dot = np.einsum("mbd,bd->mb", lnu[d_], full)
        s_all[d_] = dot - np.log(u_all[d_].sum(axis=-1))

    # exact float64 re-evaluation of near-max candidates
    out = np.zeros((2, B))
    st = state  # (B, D) float64
    for d_ in range(2):
        sm = s_all[d_]  # (MPAD, B)
        thr = sm.max(axis=0) - DELTA
        cand = np.where((sm >= thr[None, :]).any(axis=1))[0]
        bsel = bits[cand]  # (K, D)
        w1, b1, w2, b2 = dirs[d_]

        def mlp(x):
            h = np.maximum(x @ w1.T + b1, 0.0)
            lg = h @ w2.T + b2
            lg = lg - lg.max(axis=-1, keepdims=True)
            e = np.exp(lg)
            return e / e.sum(axis=-1, keepdims=True)

        full = mlp(st)  # (B, D)
        sa = mlp(st[None, :, :] * bsel[:, None, :])  # (K, B, D)
        sb = mlp(st[None, :, :] * (1.0 - bsel)[:, None, :])
        mix = 0.5 * (sa + sb)
        kl = (full[None] * (np.log2(full[None]) - np.log2(mix))).sum(-1)  # (K, B)
        out[d_] = kl.min(axis=0)
    return out.astype(np.float32)


if __name__ == "__main__":
    import reference

    inp = reference.setup_inputs()
    inp = {k: np.asarray(v) for k, v in inp.items()}
    out = kernel(**inp)
    print(out)

